# revision 15
# baseline (speedup 1.0000x reference)
"""GCN layer (normalized adjacency aggregation) on 8 Trainium2 NeuronCores.

Algorithm (row-sharded, minimal communication):
    a_hat = A + I  (identity folded into the shard on the host)
    deg[i] = sum_j a_hat[i, j]              -> per-core PE ones-matmul over its rows
    dinv = deg ** -0.5                      -> sqrt + reciprocal, AllGather (4KB)
    sup = x @ W.T + b                       -> computed redundantly per core (tiny)
    S = dinv[:, None] * sup
    out[i, :] = dinv[i] * (a_hat[i, :] @ S) -> accumulating matmul over the row block

Each core receives its row block of a_hat TRANSPOSED ([N, N/8], contraction dim
on partitions) so both the degree pass and the aggregation pass stream it as the
matmul moving operand with no on-chip transposes.  The block is streamed twice
(degree, then aggregation); the last NRES tiles of the first pass stay resident
in SBUF and are not re-read.  Matmuls run in float32r mode (single-pass fp32
multiply, 4x the instruction rate of full fp32 LOW_HIGH).
"""

import numpy as np
from contextlib import ExitStack

N = 8192
F = 128
NCORES = 8
RPC = N // NCORES  # 1024 rows per core
P = 128            # partitions
JT = N // P        # 64 column tiles of the (transposed) block

# SBUF knobs.  Per-partition budget ~192KB: sup_all 32KB + misc ~16KB +
# resident 10 pairs x 8KB + 3-pair phase-A stream + 4-pair reload pool.
NRES_PAIRS = 10
STREAM_PAIRS = 3
RELOAD_PAIRS = 4


def build_module(n=N, f=F, ncores=NCORES, nres_pairs=NRES_PAIRS,
                 stream_pairs=STREAM_PAIRS, reload_pairs=None,
                 use_f32r=True, debug_taps=False):
    """Build and compile the SPMD Bass module (same program on every core)."""
    from concourse import bass, bacc, tile

    mybir = bass.mybir
    dt = mybir.dt.float32
    dtr = mybir.dt.float32r if use_f32r else dt

    if reload_pairs is None:
        reload_pairs = RELOAD_PAIRS
    rpc = n // ncores
    jt = n // P
    pairs = jt // 2
    j_stream_pairs = pairs - nres_pairs
    j_stream = 2 * j_stream_pairs
    per_rank = rpc // P

    nc = bacc.Bacc(
        "TRN2",
        target_bir_lowering=False,
        debug=False,
        enable_asserts=False,
        num_devices=ncores,
    )

    at_d = nc.dram_tensor("at", [n, rpc], dtr, kind="ExternalInput")
    xt_d = nc.dram_tensor("xt", [f, n], dtr, kind="ExternalInput")
    wt_d = nc.dram_tensor("wt", [f, f], dtr, kind="ExternalInput")
    b_d = nc.dram_tensor("bias", [1, f], dt, kind="ExternalInput")
    ones_r_d = nc.dram_tensor("ones_r", [1, P], dt, kind="ExternalInput")
    ones_c_d = nc.dram_tensor("ones_c", [P, 1], dtr, kind="ExternalInput")
    out_d = nc.dram_tensor("out_t", [f, rpc], dt, kind="ExternalOutput")
    if debug_taps:
        tap_sqrt_d = nc.dram_tensor("tap_sqrt", [1, rpc], dt, kind="ExternalOutput")
        tap_dinv_d = nc.dram_tensor("tap_dinv", [P, n // P], dt, kind="ExternalOutput")
        tap_s_d = nc.dram_tensor("tap_s", [P, (n // P) * f], dt, kind="ExternalOutput")

    with tile.TileContext(nc) as tc, ExitStack() as ctx:
        cpool = ctx.enter_context(tc.tile_pool(name="const", bufs=1))
        wt_sb = cpool.tile([f, f], dtr, name="wt_sb")
        bias_sb = cpool.tile([1, f], dt, name="bias_sb")
        ones_r = cpool.tile([1, P], dt, name="ones_r")
        ones_c = cpool.tile([P, 1], dtr, name="ones_c")
        b_rep = cpool.tile([P, f], dt, name="b_rep")
        sup_all = cpool.tile([P, jt * f], dtr, name="sup_all")
        dinv_sb = cpool.tile([P, jt], dt, name="dinv_sb")
        dinv_l = cpool.tile([1, rpc], dt, name="dinv_l")
        dinv_rep = cpool.tile([P, rpc], dt, name="dinv_rep")
        out_sb = cpool.tile([P, rpc], dt, name="out_sb")

        dram = ctx.enter_context(tc.tile_pool(name="dram", bufs=1, space="DRAM"))
        ag_in = dram.tile([1, rpc], dt, name="ag_in")
        ag_out = dram.tile([ncores, rpc], dt, name="ag_out")

        nc.gpsimd.dma_start(wt_sb[:], wt_d[:])
        nc.gpsimd.dma_start(bias_sb[:], b_d[:])
        nc.gpsimd.dma_start(ones_r[:], ones_r_d[:])
        nc.gpsimd.dma_start(ones_c[:], ones_c_d[:])

        apool_res = ctx.enter_context(tc.tile_pool(name="a_res", bufs=max(nres_pairs, 1)))
        apool_str = ctx.enter_context(tc.tile_pool(name="a_str", bufs=stream_pairs))
        apool_rld = ctx.enter_context(tc.tile_pool(name="a_rld", bufs=reload_pairs))
        xpool = ctx.enter_context(tc.tile_pool(name="xts", bufs=4))
        res_tiles = {}   # pair index -> tile [P, 2*rpc]

        def load_pair(pool, pj, tag, nm, eng=None):
            t = pool.tile([P, 2 * rpc], dtr, name=nm, tag=tag)
            src = at_d[pj * 2 * P:(pj + 1) * 2 * P, :].rearrange(
                "(h p) i -> p h i", p=P)
            (eng or nc.sync).dma_start(t[:], src)
            return t

        # ---- Phase A: support = x @ W.T + b, and degree row sums ----
        with (
            tc.tile_pool(name="psum_s", bufs=2, space="PSUM") as psum_s,
            tc.tile_pool(name="psum_r", bufs=1, space="PSUM") as psum_r,
        ):
            # bias broadcast via outer product: ones_r.T @ bias -> [P, f]
            pb = psum_s.tile([P, f], dt, name="pb", tag="pb")
            nc.tensor.matmul(pb[:], ones_r[:], bias_sb[:], start=True, stop=True)
            nc.vector.tensor_copy(b_rep[:], pb[:])

            for j in range(jt):
                xts = xpool.tile([f, f], dtr, name=f"xts{j}", tag="xts")
                nc.gpsimd.dma_start(xts[:], xt_d[:, j * f:(j + 1) * f])
                ps = psum_s.tile([P, f], dt, name=f"ps{j}", tag="ps")
                nc.tensor.matmul(ps[:], xts[:], wt_sb[:], start=True, stop=True)
                nc.vector.tensor_add(sup_all[:, j * f:(j + 1) * f], ps[:], b_rep[:])

            # degree: accumulate ones_c.T @ a_tile into [1, rpc]
            pr = psum_r.tile([1, rpc], dt, name="pr")
            for pj in range(pairs):
                if pj >= j_stream_pairs:
                    a_t = load_pair(apool_res, pj, "ares", f"a{pj}", eng=nc.scalar)
                    res_tiles[pj] = a_t
                else:
                    a_t = load_pair(apool_str, pj, "astr", f"a{pj}")
                for half in range(2):
                    j = 2 * pj + half
                    for h in range(0, rpc, 512):
                        w = min(512, rpc - h)
                        nc.tensor.matmul(
                            pr[:, h:h + w], ones_c[:],
                            a_t[:, half * rpc + h:half * rpc + h + w],
                            start=(j == 0), stop=(j == jt - 1),
                        )

            nc.scalar.sqrt(dinv_l[:], pr[:])

        nc.vector.reciprocal(dinv_l[:], dinv_l[:])

        # row-scale broadcast dinv_rep = ones_r.T x dinv_l (local; overlaps AG)
        with tc.tile_pool(name="psum_d", bufs=1, space="PSUM") as psum_d:
            pd = psum_d.tile([f, rpc], dt, name="pd")
            for h in range(0, rpc, 512):
                w = min(512, rpc - h)
                nc.tensor.matmul(
                    pd[:, h:h + w], ones_r[:], dinv_l[:, h:h + w],
                    start=True, stop=True,
                )
            nc.vector.tensor_copy(dinv_rep[:], pd[:])

        # ---- Phase B: AllGather dinv across the cores ----
        nc.gpsimd.dma_start(ag_in[:], dinv_l[:])
        nc.gpsimd.collective_compute(
            "AllGather",
            mybir.AluOpType.bypass,
            replica_groups=[list(range(ncores))],
            ins=[ag_in.opt()],
            outs=[ag_out.opt()],
        )
        # dinv partition-major: dinv_sb[:, a*ncores + r0] = dinv[r0*rpc + a*P : +P]
        engs = [nc.scalar, nc.gpsimd]
        for a in range(per_rank):
            src = ag_out[:, a * P:(a + 1) * P].rearrange("r p -> p r")
            engs[a % len(engs)].dma_start(
                dinv_sb[:, a * ncores:(a + 1) * ncores], src)

        def dinv_col(j):  # column of dinv_sb holding dinv for j-tile j
            r0, a = j // per_rank, j % per_rank
            return a * ncores + r0

        # ---- Phase C: scale support columns: S[j, :] = dinv[j] * sup[j, :] ----
        # resident-pair columns first (phase D consumes them first)
        for j in list(range(j_stream, jt)) + list(range(j_stream)):
            sl = slice(j * f, (j + 1) * f)
            c = dinv_col(j)
            nc.vector.tensor_scalar_mul(sup_all[:, sl], sup_all[:, sl],
                                        dinv_sb[:, c:c + 1])

        if debug_taps:
            nc.scalar.dma_start(tap_sqrt_d[:], sqrt_t[:])
            nc.scalar.dma_start(tap_dinv_d[:], dinv_sb[:])
            nc.scalar.dma_start(tap_s_d[:], sup_all[:].bitcast(dt))

        # ---- Phase D: out.T = sum_j S[j].T @ a_hat.T[j] (accumulate over j) ----
        with tc.tile_pool(name="psum_o", bufs=1, space="PSUM") as psum_o:
            po = psum_o.tile([f, rpc], dt, name="po")
            order = list(range(j_stream_pairs, pairs)) + list(range(j_stream_pairs))
            for idx, pj in enumerate(order):
                if pj in res_tiles:
                    a_t = res_tiles[pj]
                else:
                    a_t = load_pair(apool_rld, pj, "arld", f"a2_{pj}")
                for half in range(2):
                    j = 2 * pj + half
                    sl = slice(j * f, (j + 1) * f)
                    # start/stop are per psum REGION: first/last writer of
                    # each po[:, h] slice carries the flag
                    first = (idx == 0 and half == 0)
                    last = (idx == len(order) - 1 and half == 1)
                    for h in range(0, rpc, 512):
                        w = min(512, rpc - h)
                        nc.tensor.matmul(
                            po[:, h:h + w], sup_all[:, sl],
                            a_t[:, half * rpc + h:half * rpc + h + w],
                            start=first, stop=last,
                        )

            # ---- Phase E: out = dinv[i] * out ----
            nc.vector.tensor_mul(out_sb[:], po[:], dinv_rep[:])

        nc.scalar.dma_start(out_d[:], out_sb[:])

    nc.compile()
    return nc


_module_cache = {}


def _get_module():
    if "nc" not in _module_cache:
        nc = build_module()
        from concourse.bass_interp import get_hw_module

        nc.m = get_hw_module(nc.m)
        _module_cache["nc"] = nc
    return _module_cache["nc"]


def make_in_maps(x, adjacency, W, b, n=N, f=F, ncores=NCORES):
    rpc = n // ncores
    x = np.asarray(x, dtype=np.float32)
    adjacency = np.asarray(adjacency, dtype=np.float32)
    W = np.asarray(W, dtype=np.float32)
    b = np.asarray(b, dtype=np.float32)
    xt = np.ascontiguousarray(x.T)
    wt = np.ascontiguousarray(W.T)
    bias = np.ascontiguousarray(b.reshape(1, f))
    ones_r = np.ones((1, P), dtype=np.float32)
    ones_c = np.ones((P, 1), dtype=np.float32)
    in_maps = []
    for c in range(ncores):
        at = np.ascontiguousarray(adjacency[c * rpc:(c + 1) * rpc, :].T)
        # fold a_hat = A + I into the shard: global row c*rpc+i, column c*rpc+i
        at[c * rpc + np.arange(rpc), np.arange(rpc)] += 1.0
        in_maps.append({
            "at": at, "xt": xt, "wt": wt, "bias": bias,
            "ones_r": ones_r, "ones_c": ones_c,
        })
    return in_maps


def kernel(x, adjacency, W, b):
    from concourse.bass_utils import run_bass_kernel_spmd

    nc = _get_module()
    in_maps = make_in_maps(x, adjacency, W, b)
    res = run_bass_kernel_spmd(nc, in_maps, core_ids=list(range(NCORES)))
    out = np.empty((N, F), dtype=np.float32)
    for c in range(NCORES):
        out[c * RPC:(c + 1) * RPC, :] = res.results[c]["out_t"].T
    return out
